# revision 7
# baseline (speedup 1.0000x reference)
"""Trainium2 Bass kernel for nn_ActorNetwork (2-layer GCN + actor head).

Self-contained: hardcodes all shapes/sharding (8 NeuronCores).

Strategy:
  - Shard dst nodes (= graphs) contiguously across 8 cores (10240 nodes =
    256 graphs per core).
  - Edges are random over the full node set; each layer gathers source rows
    with gpsimd dma_gather (edges sorted by (src-chunk, dst-tile) on host),
    aggregates per 128-dst tile with one-hot matmuls on TensorE (bf16).
  - Self-loops bypass the gather (sequential stream + PE transpose).
  - Between layers: AllGather of the dinv-prescaled h1@W2 ("m2s") so every
    core can gather any source row of layer 2.
  - Head: host rows are static (first 13 of each 40); strided SBUF->SBUF
    DMAs build the [13*64, graphs] lhsT; dst-side dinv/bias/relu applied on
    the selected slots only; f32 GEMM + softmax.
"""
import sys
import hashlib

sys.path.insert(0, "/opt/trn_rl_repo")

import numpy as np
import ml_dtypes
from contextlib import ExitStack

from concourse import bass, mybir, tile, bass_utils, bacc
from concourse.masks import make_identity

F32 = mybir.dt.float32
BF16 = mybir.dt.bfloat16
I16 = mybir.dt.int16
I32 = mybir.dt.int32

N_CORES = 8
N = 81920
NL = N // N_CORES          # 10240 nodes per core
IN_DIM = 128
H1 = 256
H2 = 64
GRAPH = 40
NH = 13
ACT = 145
TILES = NL // 128          # 80 dst tiles per core
GPC = NL // GRAPH          # 256 graphs per core
CHUNK = 32768
CHUNKS = [(0, 32768), (32768, 32768), (65536, 16384)]
NCH = 3
CALL_G = 16                # groups (of 128 idxs) per dma_gather call
SENT = 300.0               # sentinel dst value for padding slots


# ---------------------------------------------------------------- host prep

def _prep(ei):
    src = ei[0].astype(np.int64)
    dst = ei[1].astype(np.int64)
    deg = np.bincount(dst, minlength=N).astype(np.float64) + 1.0
    dinv = (1.0 / np.sqrt(deg)).astype(np.float32)
    coef = (dinv[src] * dinv[dst]).astype(np.float32)

    core = dst // NL
    t_of = (dst % NL) // 128
    c_of = src // CHUNK
    idxl = src % CHUNK
    dloc = (dst % 128).astype(np.float32)

    counts = np.zeros((N_CORES, NCH, TILES), np.int64)
    np.add.at(counts, (core, c_of, t_of), 1)
    cmax = counts.max(axis=0)                       # [3, 80]
    G = np.ceil(cmax / 128).astype(np.int64)        # groups per segment
    seg_off = np.zeros((NCH, TILES), np.int64)
    off = 0
    for c in range(NCH):
        for t in range(TILES):
            seg_off[c, t] = off
            off += 128 * int(G[c, t])
    L = int(off)

    idx_all = np.zeros((N_CORES, L), np.int16)
    dstv_all = np.full((N_CORES, L), SENT, np.float32)
    coef_all = np.zeros((N_CORES, L), np.float32)
    for r in range(N_CORES):
        m = core == r
        sc, st = c_of[m], t_of[m]
        si, sd, scf = idxl[m], dloc[m], coef[m]
        order = np.lexsort((st, sc))
        sc, st = sc[order], st[order]
        si, sd, scf = si[order], sd[order], scf[order]
        key = sc * TILES + st
        change = np.r_[True, key[1:] != key[:-1]]
        starts = np.flatnonzero(change)
        runid = np.cumsum(change) - 1
        within = np.arange(len(key)) - starts[runid]
        base_run = seg_off[sc[starts], st[starts]]
        pos = base_run[runid] + within
        idx_all[r, pos] = si.astype(np.int16)
        dstv_all[r, pos] = sd
        coef_all[r, pos] = scf

    idx_sb = np.stack([
        np.tile(idx_all[r].reshape(-1, 16).T, (8, 1)) for r in range(N_CORES)
    ])                                               # [8, 128, L/16]
    # host-built one-hot tensors [128(slot%128), L/128, 128(dst)]:
    #   oh1 values = coef (layer 1), oh2 values = 1.0 (layer 2); pads -> 0
    slots = np.arange(L)
    p_of = (slots % 128)
    col_of = slots // 128
    oh1 = np.zeros((N_CORES, 128, L // 128, 128), np.float32)
    oh2 = np.zeros((N_CORES, 128, L // 128, 128), np.float32)
    for r in range(N_CORES):
        valid = dstv_all[r] != SENT
        pv, cv = p_of[valid], col_of[valid]
        dv = dstv_all[r][valid].astype(np.int64)
        oh1[r, pv, cv, dv] = coef_all[r][valid]
        oh2[r, pv, cv, dv] = 1.0

    dinv_l = dinv.reshape(N_CORES, NL)
    dinv_tiles = np.ascontiguousarray(
        dinv_l.reshape(N_CORES, TILES, 128).transpose(0, 2, 1))   # [8,128,80]
    dinv2_tiles = (dinv_tiles ** 2).astype(np.float32)

    # per-slot dst dinv for the head: hzT[p, k, g] -> host h=2k+(p>=64),
    # feat=p%64, local node g*40+h
    dinv_hz = np.zeros((N_CORES, 128, 7, GPC), np.float32)
    for k in range(7):
        for half in range(2):
            h = 2 * k + half
            if h >= NH:
                continue
            nodes = np.arange(GPC) * GRAPH + h
            dinv_hz[:, 64 * half:64 * (half + 1), k, :] = \
                dinv_l[:, nodes][:, None, :]

    # call schedule (common)
    calls = []      # (chunk, slot0, [(t, first, last), ...])
    for c in range(NCH):
        groups = []
        for t in range(TILES):
            g = int(G[c, t])
            for i in range(g):
                groups.append((t, i == 0, i == g - 1))
        gi = 0
        slot = int(seg_off[c, 0])
        while gi < len(groups):
            n = min(CALL_G, len(groups) - gi)
            calls.append((c, slot, groups[gi:gi + n]))
            slot += n * 128
            gi += n

    return dict(G=G, L=L, calls=calls, idx_sb=idx_sb, oh1=oh1, oh2=oh2,
                dinv_tiles=dinv_tiles,
                dinv2_tiles=dinv2_tiles, dinv_hz=dinv_hz)


# ---------------------------------------------------------------- builder

def _edge_pass(nc, wk, psA, calls, src_dram, elem, idxt, oh_dram,
               agg, selfT, l2):
    """Shared edge-aggregation pass for both layers."""
    open_ps = {}
    for (c, slot0, groups) in calls:
        rows0, nrows = CHUNKS[c]
        n_g = len(groups)
        gat = wk.tile([128, CALL_G, elem], F32, tag="gat", bufs=3)
        nc.gpsimd.dma_gather(
            out_ap=gat[:, 0:n_g, :],
            in_ap=src_dram[rows0:rows0 + nrows, :],
            idxs_ap=idxt[:, slot0 // 16: slot0 // 16 + n_g * 8],
            num_idxs=n_g * 128, num_idxs_reg=n_g * 128,
            elem_size=elem, single_packet=False)
        s0 = slot0 // 128
        oh = wk.tile([128, CALL_G, 128], F32, tag="oh", bufs=3)
        nc.sync.dma_start(out=oh[:, 0:n_g, :], in_=oh_dram[:, s0:s0 + n_g, :])
        M = 64 if l2 else 128
        for g, (t, first, last) in enumerate(groups):
            if first:
                open_ps[t] = psA.tile([M, 128], F32, tag="agg",
                                      name=f"aggps_c{c}_t{t}")
            ps = open_ps[t]
            nc.tensor.matmul(out=ps[:], lhsT=gat[:, g, :], rhs=oh[:, g, :],
                             start=first, stop=last)
            if last:
                sl = slice(128 * t, 128 * (t + 1))
                if c == 0:
                    nc.vector.tensor_tensor(out=agg[:, sl], in0=ps[:],
                                            in1=selfT[:, sl],
                                            op=mybir.AluOpType.add)
                else:
                    nc.vector.tensor_tensor(out=agg[:, sl], in0=agg[:, sl],
                                            in1=ps[:], op=mybir.AluOpType.add)
                del open_ps[t]


def _build(L, calls):
    nc = bacc.Bacc("TRN2", target_bir_lowering=False, debug=False,
                   num_devices=N_CORES)
    d_xfull = nc.dram_tensor("xfull", [N, IN_DIM], F32, kind="ExternalInput")
    d_xloc = nc.dram_tensor("xloc", [NL, IN_DIM], F32, kind="ExternalInput")
    d_idx = nc.dram_tensor("idx", [128, L // 16], I16, kind="ExternalInput")
    d_oh1 = nc.dram_tensor("oh1", [128, (L // 128) * 128], F32,
                           kind="ExternalInput")
    d_oh2 = nc.dram_tensor("oh2", [128, (L // 128) * 128], F32,
                           kind="ExternalInput")
    d_dinvt = nc.dram_tensor("dinvt", [128, TILES], F32, kind="ExternalInput")
    d_dinv2t = nc.dram_tensor("dinv2t", [128, TILES], F32, kind="ExternalInput")
    d_dinvhz = nc.dram_tensor("dinvhz", [128, 7 * GPC], F32, kind="ExternalInput")
    d_W1 = nc.dram_tensor("W1", [IN_DIM, H1], F32, kind="ExternalInput")
    d_b1 = nc.dram_tensor("b1", [H1, 1], F32, kind="ExternalInput")
    d_W2 = nc.dram_tensor("W2", [H1, H2], F32, kind="ExternalInput")
    d_b2hz = nc.dram_tensor("b2hz", [128, 1], F32, kind="ExternalInput")
    d_Wout = nc.dram_tensor("Wout", [NH * H2, ACT], F32, kind="ExternalInput")
    d_bout = nc.dram_tensor("bout", [1, ACT], F32, kind="ExternalInput")
    d_out = nc.dram_tensor("out", [GPC, ACT], F32, kind="ExternalOutput")

    with tile.TileContext(nc) as tc, ExitStack() as top:
        perm = top.enter_context(tc.tile_pool(name="perm", bufs=1))
        dram = top.enter_context(tc.tile_pool(name="dram", bufs=1, space="DRAM"))

        # ---- persistent tiles
        idxt = perm.tile([128, L // 16], I16)
        nc.sync.dma_start(out=idxt[:], in_=d_idx[:])
        dinvt = perm.tile([128, TILES], F32)
        nc.sync.dma_start(out=dinvt[:], in_=d_dinvt[:])
        dinv2t = perm.tile([128, TILES], F32)
        nc.sync.dma_start(out=dinv2t[:], in_=d_dinv2t[:])
        W1sb = perm.tile([128, H1], BF16)
        nc.gpsimd.dma_start(out=W1sb[:], in_=d_W1[:])
        b1sb = perm.tile([128, 2], F32)
        nc.sync.dma_start(out=b1sb[:, 0:1], in_=d_b1[0:128, :])
        nc.sync.dma_start(out=b1sb[:, 1:2], in_=d_b1[128:256, :])
        W2sb = perm.tile([128, 2, H2], BF16)
        nc.gpsimd.dma_start(out=W2sb[:, 0, :], in_=d_W2[0:128, :])
        nc.gpsimd.dma_start(out=W2sb[:, 1, :], in_=d_W2[128:256, :])
        b2hz = perm.tile([128, 1], F32)
        nc.sync.dma_start(out=b2hz[:], in_=d_b2hz[:])
        WoutSB = perm.tile([128, 7, ACT], F32)
        for k in range(6):
            nc.sync.dma_start(out=WoutSB[:, k, :],
                              in_=d_Wout[128 * k:128 * (k + 1), :])
        nc.sync.dma_start(out=WoutSB[0:64, 6, :], in_=d_Wout[768:832, :])
        boutrep = perm.tile([128, ACT], F32)
        nc.sync.dma_start(out=boutrep[:], in_=d_bout[:].to_broadcast((128, ACT)))
        dinvhz = perm.tile([128, 7, GPC], F32)
        nc.sync.dma_start(out=dinvhz[:].rearrange("p k g -> p (k g)"),
                          in_=d_dinvhz[:])

        ident = perm.tile([128, 128], F32)
        make_identity(nc, ident[:])
        iota_i = perm.tile([128, 128], I32)
        nc.gpsimd.iota(iota_i[:], pattern=[[1, 128]], base=0,
                       channel_multiplier=0)
        iota_f = perm.tile([128, 128], F32)
        nc.vector.tensor_copy(out=iota_f[:], in_=iota_i[:])
        # ident_hi[p, j] = 1 if j == p + 64 (used to shift rows up by 64)
        ioc = perm.tile([128, 1], I32)
        nc.gpsimd.iota(ioc[:], pattern=[[1, 1]], base=64, channel_multiplier=1)
        iocf = perm.tile([128, 1], F32)
        nc.vector.tensor_copy(out=iocf[:], in_=ioc[:])
        ident_hi = perm.tile([128, 128], F32)
        nc.vector.tensor_tensor(out=ident_hi[:],
                                in0=iocf[:].to_broadcast((128, 128)),
                                in1=iota_f[:], op=mybir.AluOpType.is_equal)

        h1T = perm.tile([128, 2, NL], BF16)

        # =========================== Layer 1 ===========================
        with ExitStack() as ph1:
            mid1 = ph1.enter_context(tc.tile_pool(name="mid1", bufs=1))
            wk1 = ph1.enter_context(tc.tile_pool(name="wk1", bufs=2))
            psA = ph1.enter_context(tc.tile_pool(name="psA", bufs=4,
                                                 space="PSUM"))
            psT = ph1.enter_context(tc.tile_pool(name="psT", bufs=2,
                                                 space="PSUM"))

            agg1 = mid1.tile([128, NL], BF16)
            xTs = mid1.tile([128, NL], BF16)
            for t in range(TILES):
                xl = wk1.tile([128, 128], F32, tag="xl")
                nc.sync.dma_start(out=xl[:], in_=d_xloc[128 * t:128 * (t + 1), :])
                xls = wk1.tile([128, 128], F32, tag="xls")
                nc.vector.tensor_scalar_mul(xls[:], xl[:], dinv2t[:, t:t + 1])
                pt = psT.tile([128, 128], F32, tag="tr")
                nc.tensor.transpose(out=pt[:], in_=xls[:], identity=ident[:])
                nc.scalar.activation(out=xTs[:, 128 * t:128 * (t + 1)],
                                     in_=pt[:],
                                     func=mybir.ActivationFunctionType.Copy)

            _edge_pass(nc, wk1, psA, calls, d_xfull, IN_DIM, idxt,
                       d_oh1[:].rearrange("p (c d) -> p c d", d=128),
                       agg1, xTs, l2=False)

            with tc.tile_pool(name="psG1", bufs=2, space="PSUM") as psG:
                for m in range(2):
                    for nb in range(NL // 512):
                        pg = psG.tile([128, 512], F32, tag="g1")
                        nc.tensor.matmul(
                            out=pg[:], lhsT=W1sb[:, 128 * m:128 * (m + 1)],
                            rhs=agg1[:, 512 * nb:512 * (nb + 1)],
                            start=True, stop=True)
                        nc.scalar.activation(
                            out=h1T[:, m, 512 * nb:512 * (nb + 1)], in_=pg[:],
                            func=mybir.ActivationFunctionType.Relu,
                            bias=b1sb[:, m:m + 1], scale=1.0)

        # ================== GEMM2 + m2s + AllGather ====================
        m2sl = dram.tile([NL, H2], F32)
        m2sf = dram.tile([N, H2], F32, addr_space="Shared")
        with ExitStack() as ph2:
            midA = ph2.enter_context(tc.tile_pool(name="midA", bufs=1))
            m2sTs = midA.tile([64, NL], BF16)
            agg2 = midA.tile([64, NL], F32)
            with ExitStack() as ph2a:
                mid2 = ph2a.enter_context(tc.tile_pool(name="mid2", bufs=1))
                psG2 = ph2a.enter_context(tc.tile_pool(name="psG2", bufs=2,
                                                       space="PSUM"))
                psT2 = ph2a.enter_context(tc.tile_pool(name="psT2", bufs=2,
                                                       space="PSUM"))
                stage = mid2.tile([128, TILES, H2], F32)
                for t in range(TILES):
                    pg = psG2.tile([128, H2], F32, tag="g2")
                    for m in range(2):
                        nc.tensor.matmul(
                            out=pg[:], lhsT=h1T[:, m, 128 * t:128 * (t + 1)],
                            rhs=W2sb[:, m, :], start=(m == 0), stop=(m == 1))
                    nc.vector.tensor_scalar_mul(stage[:, t, :], pg[:],
                                                dinvt[:, t:t + 1])
                for t in range(TILES):
                    pt = psT2.tile([64, 128], F32, tag="tr2")
                    nc.tensor.transpose(out=pt[:], in_=stage[:, t, :],
                                        identity=ident[:])
                    nc.scalar.activation(
                        out=m2sTs[:, 128 * t:128 * (t + 1)], in_=pt[:],
                        func=mybir.ActivationFunctionType.Copy)
                nc.sync.dma_start(
                    out=m2sl[:].rearrange("(t p) f -> p t f", p=128),
                    in_=stage[:])
            nc.gpsimd.collective_compute(
                "AllGather", mybir.AluOpType.bypass,
                replica_groups=[list(range(N_CORES))],
                ins=[m2sl[:].opt()], outs=[m2sf[:].opt()])

            # ========================= Layer 2 =========================
            with ExitStack() as ph3:
                wk2 = ph3.enter_context(tc.tile_pool(name="wk2", bufs=2))
                psA2 = ph3.enter_context(tc.tile_pool(name="psA2", bufs=4,
                                                      space="PSUM"))
                _edge_pass(nc, wk2, psA2, calls, m2sf, H2, idxt,
                           d_oh2[:].rearrange("p (c d) -> p c d", d=128),
                           agg2, m2sTs, l2=True)

            # ===================== actor head ==========================
            with ExitStack() as ph4:
                mid4 = ph4.enter_context(tc.tile_pool(name="mid4", bufs=1))
                wk4 = ph4.enter_context(tc.tile_pool(name="wk4", bufs=2))
                psF = ph4.enter_context(tc.tile_pool(name="psF", bufs=2,
                                                     space="PSUM"))
                hzT = mid4.tile([128, 7, GPC], F32)
                h2r = agg2[:].rearrange("p (g q) -> p q g", q=GRAPH)
                for k in range(7):
                    pk = psF.tile([128, GPC], F32, tag="hz", name=f"hzps{k}")
                    nc.tensor.matmul(out=pk[:], lhsT=ident[0:64, :],
                                     rhs=h2r[:, 2 * k, :],
                                     start=True, stop=(k == 6))
                    if k < 6:
                        nc.tensor.matmul(out=pk[:], lhsT=ident_hi[0:64, :],
                                         rhs=h2r[:, 2 * k + 1, :],
                                         start=False, stop=True)
                    nc.vector.tensor_tensor(out=hzT[:, k, :], in0=pk[:],
                                            in1=dinvhz[:, k, :],
                                            op=mybir.AluOpType.mult)
                nc.scalar.activation(out=hzT[:].rearrange("p k g -> p (k g)"),
                                     in_=hzT[:].rearrange("p k g -> p (k g)"),
                                     func=mybir.ActivationFunctionType.Relu,
                                     bias=b2hz[:, 0:1], scale=1.0)
                for m in range(GPC // 128):
                    pf = psF.tile([128, ACT], F32, tag="fin")
                    for k in range(6):
                        nc.tensor.matmul(
                            out=pf[:], lhsT=hzT[:, k, 128 * m:128 * (m + 1)],
                            rhs=WoutSB[:, k, :], start=(k == 0), stop=False)
                    nc.tensor.matmul(
                        out=pf[:], lhsT=hzT[0:64, 6, 128 * m:128 * (m + 1)],
                        rhs=WoutSB[0:64, 6, :], start=False, stop=True)
                    nc.vector.tensor_tensor(out=pf[:], in0=pf[:],
                                            in1=boutrep[:],
                                            op=mybir.AluOpType.add)
                    mx = wk4.tile([128, 1], F32, tag="mx")
                    nc.vector.tensor_reduce(out=mx[:], in_=pf[:],
                                            axis=mybir.AxisListType.X,
                                            op=mybir.AluOpType.max)
                    nmx = wk4.tile([128, 1], F32, tag="nmx")
                    nc.vector.tensor_scalar_mul(nmx[:], mx[:], -1.0)
                    esb = wk4.tile([128, ACT], F32, tag="esb")
                    nc.scalar.activation(out=esb[:], in_=pf[:],
                                         func=mybir.ActivationFunctionType.Exp,
                                         bias=nmx[:, 0:1], scale=1.0)
                    ssum = wk4.tile([128, 1], F32, tag="ssum")
                    nc.vector.tensor_reduce(out=ssum[:], in_=esb[:],
                                            axis=mybir.AxisListType.X,
                                            op=mybir.AluOpType.add)
                    rcp = wk4.tile([128, 1], F32, tag="rcp")
                    nc.vector.reciprocal(out=rcp[:], in_=ssum[:])
                    osb = wk4.tile([128, ACT], F32, tag="osb")
                    nc.vector.tensor_scalar_mul(osb[:], esb[:], rcp[:, 0:1])
                    nc.sync.dma_start(out=d_out[128 * m:128 * (m + 1), :],
                                      in_=osb[:])

    nc.compile()
    return nc


# ---------------------------------------------------------------- entry

_CACHE = {}


def _get(x, ei):
    key = hashlib.sha1(ei.tobytes()).hexdigest()
    if key not in _CACHE:
        meta = _prep(ei)
        nc = _build(meta["L"], meta["calls"])
        _CACHE[key] = (meta, nc)
    return _CACHE[key]


def _in_maps(meta, x, W1, b1, W2, b2, Wout, bout):
    b2t = np.tile(np.asarray(b2, np.float32).reshape(H2), 2).reshape(128, 1)
    maps = []
    for r in range(N_CORES):
        maps.append({
            "xfull": x,
            "xloc": np.ascontiguousarray(x[r * NL:(r + 1) * NL, :]),
            "idx": np.ascontiguousarray(meta["idx_sb"][r]),
            "oh1": np.ascontiguousarray(
                meta["oh1"][r].reshape(128, -1)),
            "oh2": np.ascontiguousarray(
                meta["oh2"][r].reshape(128, -1)),
            "dinvt": np.ascontiguousarray(meta["dinv_tiles"][r]),
            "dinv2t": np.ascontiguousarray(meta["dinv2_tiles"][r]),
            "dinvhz": np.ascontiguousarray(
                meta["dinv_hz"][r].reshape(128, 7 * GPC)),
            "W1": np.ascontiguousarray(W1, np.float32),
            "b1": np.ascontiguousarray(b1, np.float32).reshape(H1, 1),
            "W2": np.ascontiguousarray(W2, np.float32),
            "b2hz": b2t,
            "Wout": np.ascontiguousarray(Wout, np.float32),
            "bout": np.ascontiguousarray(bout, np.float32).reshape(1, ACT),
        })
    return maps


def kernel(x, ei, W1, b1, W2, b2, Wout, bout, _trace=False):
    x = np.ascontiguousarray(x, np.float32)
    ei = np.ascontiguousarray(ei, np.int32)
    meta, nc = _get(x, ei)
    maps = _in_maps(meta, x, W1, b1, W2, b2, Wout, bout)
    res = bass_utils.run_bass_kernel_spmd(
        nc, maps, core_ids=list(range(N_CORES)), trace=_trace)
    out = np.concatenate([res.results[r]["out"] for r in range(N_CORES)],
                         axis=0).astype(np.float32)
    if _trace:
        return out, res.exec_time_ns
    return out


def install_profile_hook():
    import types
    sys.path.insert(0, "/root/.axon_site")
    import trn_agent_boot.trn_boot as _tb
    import antenv
    if "antenv.axon_hooks" not in sys.modules:
        _mod = types.ModuleType("antenv.axon_hooks")
        _h = [None]
        _mod.set_axon_ntff_profile_hook = lambda h: _h.__setitem__(0, h)
        _mod.get_axon_ntff_profile_hook = lambda: _h[0]
        sys.modules["antenv.axon_hooks"] = _mod
        antenv.axon_hooks = _mod
        _mod.set_axon_ntff_profile_hook(
            _tb._ntff_profile_via_ctypes("/opt/axon/libaxon_pjrt.so"))


# revision 8
# speedup vs baseline: 1.0425x; 1.0425x over previous
"""Trainium2 Bass kernel for nn_ActorNetwork (2-layer GCN + actor head).

Self-contained: hardcodes all shapes/sharding (8 NeuronCores).

Strategy:
  - Shard dst nodes (= graphs) contiguously across 8 cores (10240 nodes =
    256 graphs per core).
  - Edges are random over the full node set; each layer gathers source rows
    with gpsimd dma_gather (edges sorted by (src-chunk, dst-tile) on host),
    aggregates per 128-dst tile with one-hot matmuls on TensorE (bf16).
  - Self-loops bypass the gather (sequential stream + PE transpose).
  - Between layers: AllGather of the dinv-prescaled h1@W2 ("m2s") so every
    core can gather any source row of layer 2.
  - Head: host rows are static (first 13 of each 40); strided SBUF->SBUF
    DMAs build the [13*64, graphs] lhsT; dst-side dinv/bias/relu applied on
    the selected slots only; f32 GEMM + softmax.
"""
import sys
import hashlib

sys.path.insert(0, "/opt/trn_rl_repo")

import numpy as np
import ml_dtypes
from contextlib import ExitStack

from concourse import bass, mybir, tile, bass_utils, bacc
from concourse.masks import make_identity

F32 = mybir.dt.float32
BF16 = mybir.dt.bfloat16
I16 = mybir.dt.int16
I32 = mybir.dt.int32

N_CORES = 8
N = 81920
NL = N // N_CORES          # 10240 nodes per core
IN_DIM = 128
H1 = 256
H2 = 64
GRAPH = 40
NH = 13
ACT = 145
TILES = NL // 128          # 80 dst tiles per core
GPC = NL // GRAPH          # 256 graphs per core
CHUNK = 32768
CHUNKS = [(0, 32768), (32768, 32768), (65536, 16384)]
NCH = 3
CALL_G = 16                # groups (of 128 idxs) per dma_gather call
SENT = 300.0               # sentinel dst value for padding slots


# ---------------------------------------------------------------- host prep

def _prep(ei):
    src = ei[0].astype(np.int64)
    dst = ei[1].astype(np.int64)
    deg = np.bincount(dst, minlength=N).astype(np.float64) + 1.0
    dinv = (1.0 / np.sqrt(deg)).astype(np.float32)
    coef = (dinv[src] * dinv[dst]).astype(np.float32)

    core = dst // NL
    t_of = (dst % NL) // 128
    c_of = src // CHUNK
    idxl = src % CHUNK
    dloc = (dst % 128).astype(np.float32)

    counts = np.zeros((N_CORES, NCH, TILES), np.int64)
    np.add.at(counts, (core, c_of, t_of), 1)
    cmax = counts.max(axis=0)                       # [3, 80]
    G = np.ceil(cmax / 128).astype(np.int64)        # groups per segment
    seg_off = np.zeros((NCH, TILES), np.int64)
    off = 0
    for c in range(NCH):
        for t in range(TILES):
            seg_off[c, t] = off
            off += 128 * int(G[c, t])
    L = int(off)

    idx_all = np.zeros((N_CORES, L), np.int16)
    dstv_all = np.full((N_CORES, L), SENT, np.float32)
    coef_all = np.zeros((N_CORES, L), np.float32)
    for r in range(N_CORES):
        m = core == r
        sc, st = c_of[m], t_of[m]
        si, sd, scf = idxl[m], dloc[m], coef[m]
        order = np.lexsort((st, sc))
        sc, st = sc[order], st[order]
        si, sd, scf = si[order], sd[order], scf[order]
        key = sc * TILES + st
        change = np.r_[True, key[1:] != key[:-1]]
        starts = np.flatnonzero(change)
        runid = np.cumsum(change) - 1
        within = np.arange(len(key)) - starts[runid]
        base_run = seg_off[sc[starts], st[starts]]
        pos = base_run[runid] + within
        idx_all[r, pos] = si.astype(np.int16)
        dstv_all[r, pos] = sd
        coef_all[r, pos] = scf

    idx_sb = np.stack([
        np.tile(idx_all[r].reshape(-1, 16).T, (8, 1)) for r in range(N_CORES)
    ])                                               # [8, 128, L/16]
    # host-built one-hot tensors [128(slot%128), L/128, 128(dst)]:
    #   oh1 values = coef (layer 1), oh2 values = 1.0 (layer 2); pads -> 0
    slots = np.arange(L)
    p_of = (slots % 128)
    col_of = slots // 128
    oh1 = np.zeros((N_CORES, 128, L // 128, 128), ml_dtypes.bfloat16)
    oh2 = np.zeros((N_CORES, 128, L // 128, 128), ml_dtypes.bfloat16)
    for r in range(N_CORES):
        valid = dstv_all[r] != SENT
        pv, cv = p_of[valid], col_of[valid]
        dv = dstv_all[r][valid].astype(np.int64)
        oh1[r, pv, cv, dv] = coef_all[r][valid].astype(ml_dtypes.bfloat16)
        oh2[r, pv, cv, dv] = 1.0

    dinv_l = dinv.reshape(N_CORES, NL)
    dinv_tiles = np.ascontiguousarray(
        dinv_l.reshape(N_CORES, TILES, 128).transpose(0, 2, 1))   # [8,128,80]
    dinv2_tiles = (dinv_tiles ** 2).astype(np.float32)

    # per-slot dst dinv for the head: hzT[p, k, g] -> host h=2k+(p>=64),
    # feat=p%64, local node g*40+h
    dinv_hz = np.zeros((N_CORES, 128, 7, GPC), np.float32)
    for k in range(7):
        for half in range(2):
            h = 2 * k + half
            if h >= NH:
                continue
            nodes = np.arange(GPC) * GRAPH + h
            dinv_hz[:, 64 * half:64 * (half + 1), k, :] = \
                dinv_l[:, nodes][:, None, :]

    # call schedule (common)
    calls = []      # (chunk, slot0, [(t, first, last), ...])
    for c in range(NCH):
        groups = []
        for t in range(TILES):
            g = int(G[c, t])
            for i in range(g):
                groups.append((t, i == 0, i == g - 1))
        gi = 0
        slot = int(seg_off[c, 0])
        while gi < len(groups):
            n = min(CALL_G, len(groups) - gi)
            calls.append((c, slot, groups[gi:gi + n]))
            slot += n * 128
            gi += n

    return dict(G=G, L=L, calls=calls, idx_sb=idx_sb, oh1=oh1, oh2=oh2,
                dinv_tiles=dinv_tiles,
                dinv2_tiles=dinv2_tiles, dinv_hz=dinv_hz)


# ---------------------------------------------------------------- builder

def _edge_pass(nc, wk, psA, calls, src_dram, elem, idxt, oh_dram,
               agg, selfT, l2):
    """Shared edge-aggregation pass for both layers."""
    open_ps = {}
    for (c, slot0, groups) in calls:
        rows0, nrows = CHUNKS[c]
        n_g = len(groups)
        gat = wk.tile([128, CALL_G, elem], F32, tag="gat", bufs=3)
        nc.gpsimd.dma_gather(
            out_ap=gat[:, 0:n_g, :],
            in_ap=src_dram[rows0:rows0 + nrows, :],
            idxs_ap=idxt[:, slot0 // 16: slot0 // 16 + n_g * 8],
            num_idxs=n_g * 128, num_idxs_reg=n_g * 128,
            elem_size=elem, single_packet=False)
        s0 = slot0 // 128
        oh = wk.tile([128, CALL_G, 128], BF16, tag="oh", bufs=3)
        nc.sync.dma_start(out=oh[:, 0:n_g, :], in_=oh_dram[:, s0:s0 + n_g, :])
        gatb = wk.tile([128, CALL_G, elem], BF16, tag="gatb", bufs=3)
        nc.scalar.activation(
            out=gatb[:, 0:n_g, :].rearrange("p a b -> p (a b)"),
            in_=gat[:, 0:n_g, :].rearrange("p a b -> p (a b)"),
            func=mybir.ActivationFunctionType.Copy)
        M = 64 if l2 else 128
        for g, (t, first, last) in enumerate(groups):
            if first:
                open_ps[t] = psA.tile([M, 128], F32, tag="agg",
                                      name=f"aggps_c{c}_t{t}")
            ps = open_ps[t]
            nc.tensor.matmul(out=ps[:], lhsT=gatb[:, g, :], rhs=oh[:, g, :],
                             start=first, stop=last)
            if last:
                sl = slice(128 * t, 128 * (t + 1))
                if c == 0:
                    nc.vector.tensor_tensor(out=agg[:, sl], in0=ps[:],
                                            in1=selfT[:, sl],
                                            op=mybir.AluOpType.add)
                else:
                    nc.vector.tensor_tensor(out=agg[:, sl], in0=agg[:, sl],
                                            in1=ps[:], op=mybir.AluOpType.add)
                del open_ps[t]


def _build(L, calls):
    nc = bacc.Bacc("TRN2", target_bir_lowering=False, debug=False,
                   num_devices=N_CORES)
    d_xfull = nc.dram_tensor("xfull", [N, IN_DIM], F32, kind="ExternalInput")
    d_xloc = nc.dram_tensor("xloc", [NL, IN_DIM], F32, kind="ExternalInput")
    d_idx = nc.dram_tensor("idx", [128, L // 16], I16, kind="ExternalInput")
    d_oh1 = nc.dram_tensor("oh1", [128, (L // 128) * 128], BF16,
                           kind="ExternalInput")
    d_oh2 = nc.dram_tensor("oh2", [128, (L // 128) * 128], BF16,
                           kind="ExternalInput")
    d_dinvt = nc.dram_tensor("dinvt", [128, TILES], F32, kind="ExternalInput")
    d_dinv2t = nc.dram_tensor("dinv2t", [128, TILES], F32, kind="ExternalInput")
    d_dinvhz = nc.dram_tensor("dinvhz", [128, 7 * GPC], F32, kind="ExternalInput")
    d_W1 = nc.dram_tensor("W1", [IN_DIM, H1], F32, kind="ExternalInput")
    d_b1 = nc.dram_tensor("b1", [H1, 1], F32, kind="ExternalInput")
    d_W2 = nc.dram_tensor("W2", [H1, H2], F32, kind="ExternalInput")
    d_b2hz = nc.dram_tensor("b2hz", [128, 1], F32, kind="ExternalInput")
    d_Wout = nc.dram_tensor("Wout", [NH * H2, ACT], F32, kind="ExternalInput")
    d_bout = nc.dram_tensor("bout", [1, ACT], F32, kind="ExternalInput")
    d_out = nc.dram_tensor("out", [GPC, ACT], F32, kind="ExternalOutput")

    with tile.TileContext(nc) as tc, ExitStack() as top:
        perm = top.enter_context(tc.tile_pool(name="perm", bufs=1))
        dram = top.enter_context(tc.tile_pool(name="dram", bufs=1, space="DRAM"))

        # ---- persistent tiles
        idxt = perm.tile([128, L // 16], I16)
        nc.sync.dma_start(out=idxt[:], in_=d_idx[:])
        dinvt = perm.tile([128, TILES], F32)
        nc.sync.dma_start(out=dinvt[:], in_=d_dinvt[:])
        dinv2t = perm.tile([128, TILES], F32)
        nc.sync.dma_start(out=dinv2t[:], in_=d_dinv2t[:])
        W1sb = perm.tile([128, H1], BF16)
        nc.gpsimd.dma_start(out=W1sb[:], in_=d_W1[:])
        b1sb = perm.tile([128, 2], F32)
        nc.sync.dma_start(out=b1sb[:, 0:1], in_=d_b1[0:128, :])
        nc.sync.dma_start(out=b1sb[:, 1:2], in_=d_b1[128:256, :])
        W2sb = perm.tile([128, 2, H2], BF16)
        nc.gpsimd.dma_start(out=W2sb[:, 0, :], in_=d_W2[0:128, :])
        nc.gpsimd.dma_start(out=W2sb[:, 1, :], in_=d_W2[128:256, :])
        b2hz = perm.tile([128, 1], F32)
        nc.sync.dma_start(out=b2hz[:], in_=d_b2hz[:])
        WoutSB = perm.tile([128, 7, ACT], F32)
        for k in range(6):
            nc.sync.dma_start(out=WoutSB[:, k, :],
                              in_=d_Wout[128 * k:128 * (k + 1), :])
        nc.sync.dma_start(out=WoutSB[0:64, 6, :], in_=d_Wout[768:832, :])
        boutrep = perm.tile([128, ACT], F32)
        nc.sync.dma_start(out=boutrep[:], in_=d_bout[:].to_broadcast((128, ACT)))
        dinvhz = perm.tile([128, 7, GPC], F32)
        nc.sync.dma_start(out=dinvhz[:].rearrange("p k g -> p (k g)"),
                          in_=d_dinvhz[:])

        ident = perm.tile([128, 128], F32)
        make_identity(nc, ident[:])
        iota_i = perm.tile([128, 128], I32)
        nc.gpsimd.iota(iota_i[:], pattern=[[1, 128]], base=0,
                       channel_multiplier=0)
        iota_f = perm.tile([128, 128], F32)
        nc.vector.tensor_copy(out=iota_f[:], in_=iota_i[:])
        # ident_hi[p, j] = 1 if j == p + 64 (used to shift rows up by 64)
        ioc = perm.tile([128, 1], I32)
        nc.gpsimd.iota(ioc[:], pattern=[[1, 1]], base=64, channel_multiplier=1)
        iocf = perm.tile([128, 1], F32)
        nc.vector.tensor_copy(out=iocf[:], in_=ioc[:])
        ident_hi = perm.tile([128, 128], F32)
        nc.vector.tensor_tensor(out=ident_hi[:],
                                in0=iocf[:].to_broadcast((128, 128)),
                                in1=iota_f[:], op=mybir.AluOpType.is_equal)

        h1T = perm.tile([128, 2, NL], BF16)

        # =========================== Layer 1 ===========================
        with ExitStack() as ph1:
            mid1 = ph1.enter_context(tc.tile_pool(name="mid1", bufs=1))
            wk1 = ph1.enter_context(tc.tile_pool(name="wk1", bufs=2))
            psA = ph1.enter_context(tc.tile_pool(name="psA", bufs=4,
                                                 space="PSUM"))
            psT = ph1.enter_context(tc.tile_pool(name="psT", bufs=2,
                                                 space="PSUM"))

            agg1 = mid1.tile([128, NL], BF16)
            xTs = mid1.tile([128, NL], BF16)
            for t in range(TILES):
                xl = wk1.tile([128, 128], F32, tag="xl")
                nc.sync.dma_start(out=xl[:], in_=d_xloc[128 * t:128 * (t + 1), :])
                xls = wk1.tile([128, 128], F32, tag="xls")
                nc.vector.tensor_scalar_mul(xls[:], xl[:], dinv2t[:, t:t + 1])
                pt = psT.tile([128, 128], F32, tag="tr")
                nc.tensor.transpose(out=pt[:], in_=xls[:], identity=ident[:])
                nc.scalar.activation(out=xTs[:, 128 * t:128 * (t + 1)],
                                     in_=pt[:],
                                     func=mybir.ActivationFunctionType.Copy)

            _edge_pass(nc, wk1, psA, calls, d_xfull, IN_DIM, idxt,
                       d_oh1[:].rearrange("p (c d) -> p c d", d=128),
                       agg1, xTs, l2=False)

            with tc.tile_pool(name="psG1", bufs=2, space="PSUM") as psG:
                for m in range(2):
                    for nb in range(NL // 512):
                        pg = psG.tile([128, 512], F32, tag="g1")
                        nc.tensor.matmul(
                            out=pg[:], lhsT=W1sb[:, 128 * m:128 * (m + 1)],
                            rhs=agg1[:, 512 * nb:512 * (nb + 1)],
                            start=True, stop=True)
                        nc.scalar.activation(
                            out=h1T[:, m, 512 * nb:512 * (nb + 1)], in_=pg[:],
                            func=mybir.ActivationFunctionType.Relu,
                            bias=b1sb[:, m:m + 1], scale=1.0)

        # ================== GEMM2 + m2s + AllGather ====================
        m2sl = dram.tile([NL, H2], F32)
        m2sf = dram.tile([N, H2], F32, addr_space="Shared")
        with ExitStack() as ph2:
            midA = ph2.enter_context(tc.tile_pool(name="midA", bufs=1))
            m2sTs = midA.tile([64, NL], BF16)
            agg2 = midA.tile([64, NL], F32)
            with ExitStack() as ph2a:
                mid2 = ph2a.enter_context(tc.tile_pool(name="mid2", bufs=1))
                psG2 = ph2a.enter_context(tc.tile_pool(name="psG2", bufs=2,
                                                       space="PSUM"))
                psT2 = ph2a.enter_context(tc.tile_pool(name="psT2", bufs=2,
                                                       space="PSUM"))
                stage = mid2.tile([128, TILES, H2], F32)
                for t in range(TILES):
                    pg = psG2.tile([128, H2], F32, tag="g2")
                    for m in range(2):
                        nc.tensor.matmul(
                            out=pg[:], lhsT=h1T[:, m, 128 * t:128 * (t + 1)],
                            rhs=W2sb[:, m, :], start=(m == 0), stop=(m == 1))
                    nc.vector.tensor_scalar_mul(stage[:, t, :], pg[:],
                                                dinvt[:, t:t + 1])
                for t in range(TILES):
                    pt = psT2.tile([64, 128], F32, tag="tr2")
                    nc.tensor.transpose(out=pt[:], in_=stage[:, t, :],
                                        identity=ident[:])
                    nc.scalar.activation(
                        out=m2sTs[:, 128 * t:128 * (t + 1)], in_=pt[:],
                        func=mybir.ActivationFunctionType.Copy)
                nc.sync.dma_start(
                    out=m2sl[:].rearrange("(t p) f -> p t f", p=128),
                    in_=stage[:])
            nc.gpsimd.collective_compute(
                "AllGather", mybir.AluOpType.bypass,
                replica_groups=[list(range(N_CORES))],
                ins=[m2sl[:].opt()], outs=[m2sf[:].opt()])

            # ========================= Layer 2 =========================
            with ExitStack() as ph3:
                wk2 = ph3.enter_context(tc.tile_pool(name="wk2", bufs=2))
                psA2 = ph3.enter_context(tc.tile_pool(name="psA2", bufs=4,
                                                      space="PSUM"))
                _edge_pass(nc, wk2, psA2, calls, m2sf, H2, idxt,
                           d_oh2[:].rearrange("p (c d) -> p c d", d=128),
                           agg2, m2sTs, l2=True)

            # ===================== actor head ==========================
            with ExitStack() as ph4:
                mid4 = ph4.enter_context(tc.tile_pool(name="mid4", bufs=1))
                wk4 = ph4.enter_context(tc.tile_pool(name="wk4", bufs=2))
                psF = ph4.enter_context(tc.tile_pool(name="psF", bufs=2,
                                                     space="PSUM"))
                hzT = mid4.tile([128, 7, GPC], F32)
                h2r = agg2[:].rearrange("p (g q) -> p q g", q=GRAPH)
                for k in range(7):
                    pk = psF.tile([128, GPC], F32, tag="hz", name=f"hzps{k}")
                    nc.tensor.matmul(out=pk[:], lhsT=ident[0:64, :],
                                     rhs=h2r[:, 2 * k, :],
                                     start=True, stop=(k == 6))
                    if k < 6:
                        nc.tensor.matmul(out=pk[:], lhsT=ident_hi[0:64, :],
                                         rhs=h2r[:, 2 * k + 1, :],
                                         start=False, stop=True)
                    nc.vector.tensor_tensor(out=hzT[:, k, :], in0=pk[:],
                                            in1=dinvhz[:, k, :],
                                            op=mybir.AluOpType.mult)
                nc.scalar.activation(out=hzT[:].rearrange("p k g -> p (k g)"),
                                     in_=hzT[:].rearrange("p k g -> p (k g)"),
                                     func=mybir.ActivationFunctionType.Relu,
                                     bias=b2hz[:, 0:1], scale=1.0)
                for m in range(GPC // 128):
                    pf = psF.tile([128, ACT], F32, tag="fin")
                    for k in range(6):
                        nc.tensor.matmul(
                            out=pf[:], lhsT=hzT[:, k, 128 * m:128 * (m + 1)],
                            rhs=WoutSB[:, k, :], start=(k == 0), stop=False)
                    nc.tensor.matmul(
                        out=pf[:], lhsT=hzT[0:64, 6, 128 * m:128 * (m + 1)],
                        rhs=WoutSB[0:64, 6, :], start=False, stop=True)
                    nc.vector.tensor_tensor(out=pf[:], in0=pf[:],
                                            in1=boutrep[:],
                                            op=mybir.AluOpType.add)
                    mx = wk4.tile([128, 1], F32, tag="mx")
                    nc.vector.tensor_reduce(out=mx[:], in_=pf[:],
                                            axis=mybir.AxisListType.X,
                                            op=mybir.AluOpType.max)
                    nmx = wk4.tile([128, 1], F32, tag="nmx")
                    nc.vector.tensor_scalar_mul(nmx[:], mx[:], -1.0)
                    esb = wk4.tile([128, ACT], F32, tag="esb")
                    nc.scalar.activation(out=esb[:], in_=pf[:],
                                         func=mybir.ActivationFunctionType.Exp,
                                         bias=nmx[:, 0:1], scale=1.0)
                    ssum = wk4.tile([128, 1], F32, tag="ssum")
                    nc.vector.tensor_reduce(out=ssum[:], in_=esb[:],
                                            axis=mybir.AxisListType.X,
                                            op=mybir.AluOpType.add)
                    rcp = wk4.tile([128, 1], F32, tag="rcp")
                    nc.vector.reciprocal(out=rcp[:], in_=ssum[:])
                    osb = wk4.tile([128, ACT], F32, tag="osb")
                    nc.vector.tensor_scalar_mul(osb[:], esb[:], rcp[:, 0:1])
                    nc.sync.dma_start(out=d_out[128 * m:128 * (m + 1), :],
                                      in_=osb[:])

    nc.compile()
    return nc


# ---------------------------------------------------------------- entry

_CACHE = {}


def _get(x, ei):
    key = hashlib.sha1(ei.tobytes()).hexdigest()
    if key not in _CACHE:
        meta = _prep(ei)
        nc = _build(meta["L"], meta["calls"])
        _CACHE[key] = (meta, nc)
    return _CACHE[key]


def _in_maps(meta, x, W1, b1, W2, b2, Wout, bout):
    b2t = np.tile(np.asarray(b2, np.float32).reshape(H2), 2).reshape(128, 1)
    maps = []
    for r in range(N_CORES):
        maps.append({
            "xfull": x,
            "xloc": np.ascontiguousarray(x[r * NL:(r + 1) * NL, :]),
            "idx": np.ascontiguousarray(meta["idx_sb"][r]),
            "oh1": np.ascontiguousarray(
                meta["oh1"][r].reshape(128, -1)),
            "oh2": np.ascontiguousarray(
                meta["oh2"][r].reshape(128, -1)),
            "dinvt": np.ascontiguousarray(meta["dinv_tiles"][r]),
            "dinv2t": np.ascontiguousarray(meta["dinv2_tiles"][r]),
            "dinvhz": np.ascontiguousarray(
                meta["dinv_hz"][r].reshape(128, 7 * GPC)),
            "W1": np.ascontiguousarray(W1, np.float32),
            "b1": np.ascontiguousarray(b1, np.float32).reshape(H1, 1),
            "W2": np.ascontiguousarray(W2, np.float32),
            "b2hz": b2t,
            "Wout": np.ascontiguousarray(Wout, np.float32),
            "bout": np.ascontiguousarray(bout, np.float32).reshape(1, ACT),
        })
    return maps


def kernel(x, ei, W1, b1, W2, b2, Wout, bout, _trace=False):
    x = np.ascontiguousarray(x, np.float32)
    ei = np.ascontiguousarray(ei, np.int32)
    meta, nc = _get(x, ei)
    maps = _in_maps(meta, x, W1, b1, W2, b2, Wout, bout)
    res = bass_utils.run_bass_kernel_spmd(
        nc, maps, core_ids=list(range(N_CORES)), trace=_trace)
    out = np.concatenate([res.results[r]["out"] for r in range(N_CORES)],
                         axis=0).astype(np.float32)
    if _trace:
        return out, res.exec_time_ns
    return out


def install_profile_hook():
    import types
    sys.path.insert(0, "/root/.axon_site")
    import trn_agent_boot.trn_boot as _tb
    import antenv
    if "antenv.axon_hooks" not in sys.modules:
        _mod = types.ModuleType("antenv.axon_hooks")
        _h = [None]
        _mod.set_axon_ntff_profile_hook = lambda h: _h.__setitem__(0, h)
        _mod.get_axon_ntff_profile_hook = lambda: _h[0]
        sys.modules["antenv.axon_hooks"] = _mod
        antenv.axon_hooks = _mod
        _mod.set_axon_ntff_profile_hook(
            _tb._ntff_profile_via_ctypes("/opt/axon/libaxon_pjrt.so"))
